# revision 1
# baseline (speedup 1.0000x reference)
"""GQA multi-head attention (RoPE + tanh softcap + causal mask) on 8 TRN2 cores.

Sharding: tensor-parallel over the 8 kv-head groups (1 kv head + its 4 q heads
per core).  Each core computes its Q/K/V projections from the full hidden
states, runs attention for its 4 q heads, and produces a partial output
through its row-slice of Wo; the host sums the 8 partials.

Layout strategy on device (per core):
  - hidden states are passed pre-transposed (hsT [D, S]) so projections
    produce qT/kT [d_head, seq] directly (d_head on partitions).
  - logits are computed transposed ([kcol, qrow]); the tanh softcap bounds
    them to +-30 so softmax needs no running max, and masking is a cheap
    multiply by {0,1} after exp.  The softmax denominator is accumulated on
    the vector engine and reduced across partitions with a ones-matmul; the
    per-row reciprocal is broadcast back across partitions with a K=1 matmul.
  - V is transposed back to [kcol, d_head] with the PE so A@V needs no
    transposes of the (huge) attention-weight matrix.
  - matmul operands are float32r (fast PE path, ~1e-4 relative error);
    accumulation and all softmax math stay fp32.
"""

import numpy as np

S, D, DH = 2048, 4096, 128
HQ, HKV = 32, 8
G = HQ // HKV            # q heads per core
N_CORES = 8
MULT = 0.08838834764831845
SOFTCAP = 30.0
ROPE_BASE = 10000.0
BLK = 512                # seq block (matmul moving-dim max for 4-byte dtypes)
NB = S // BLK            # 4 seq blocks
NCH = S // 128           # 16 kcol chunks
NDC = D // 128           # 32 contraction chunks for projections

_CACHE = {}


def _classify_mask(mask):
    """Per (qblock, kchunk): skip (all masked), plain (all visible), or
    mixed (transposed {0,1} tile).  active[n] = ordered [(chunk, slot)],
    slot -1 for plain; mtiles packed [n_mixed*128, BLK] fp32."""
    m = np.asarray(mask).reshape(S, S)
    active = []
    mtiles = []
    for n in range(NB):
        rows = m[n * BLK:(n + 1) * BLK]
        lst = []
        for c in range(NCH):
            sub = rows[:, c * 128:(c + 1) * 128]
            if not sub.any():
                continue
            if sub.all():
                lst.append((c, -1))
            else:
                lst.append((c, len(mtiles)))
                mtiles.append(np.ascontiguousarray(sub.T).astype(np.float32))
        active.append(lst)
    mt = (np.concatenate([t.reshape(128, BLK) for t in mtiles], axis=0)
          if mtiles else None)
    return active, mt


def _build(active, n_mixed):
    import concourse.bacc as bacc
    import concourse.mybir as mybir
    from concourse import tile
    from concourse.masks import make_identity
    from contextlib import ExitStack

    fp32 = mybir.dt.float32
    f32r = mybir.dt.float32r
    AF = mybir.ActivationFunctionType

    nc = bacc.Bacc("TRN2", target_bir_lowering=False, debug=False,
                   enable_asserts=True, num_devices=N_CORES)
    hsT = nc.dram_tensor("hsT", [D, S], f32r, kind="ExternalInput").ap()
    wq = nc.dram_tensor("wq", [D, G * DH], f32r, kind="ExternalInput").ap()
    wk = nc.dram_tensor("wk", [D, DH], f32r, kind="ExternalInput").ap()
    wv = nc.dram_tensor("wv", [D, DH], f32r, kind="ExternalInput").ap()
    wo = nc.dram_tensor("wo", [G * DH, D], f32r, kind="ExternalInput").ap()
    cosT = nc.dram_tensor("cosT", [DH, S], fp32, kind="ExternalInput").ap()
    sinT = nc.dram_tensor("sinT", [DH, S], fp32, kind="ExternalInput").ap()
    maskm = (nc.dram_tensor("maskm", [n_mixed * 128, BLK], fp32,
                            kind="ExternalInput").ap() if n_mixed else None)
    out = nc.dram_tensor("out", [S, D], fp32, kind="ExternalOutput").ap()

    with tile.TileContext(nc) as tc, ExitStack() as top:
        # per-block persistent tiles -> fine-grained cross-phase deps
        persist = top.enter_context(tc.tile_pool(name="persist", bufs=1))
        qT = [[persist.tile([DH, BLK], f32r, tag=f"qT{h}_{n}",
                            name=f"qT{h}_{n}") for n in range(NB)]
              for h in range(G)]
        kT = [persist.tile([DH, BLK], f32r, tag=f"kT{n}", name=f"kT{n}")
              for n in range(NB)]
        vnat = [persist.tile([128, DH], f32r, tag=f"vnat{c}", name=f"vnat{c}")
                for c in range(NCH)]

        # ---------------- Phase 1: QKV projections + RoPE ----------------
        with ExitStack() as ph1:
            const = ph1.enter_context(tc.tile_pool(name="p1const", bufs=1))
            wq_sb = const.tile([128, NDC, G * DH], f32r, tag="wq")
            wk_sb = const.tile([128, NDC, DH], f32r, tag="wk")
            wv_sb = const.tile([128, NDC, DH], f32r, tag="wv")
            cos_sb = const.tile([DH, S], fp32, tag="cos")
            sin_sb = const.tile([DH, S], fp32, tag="sin")
            ident = const.tile([128, 128], fp32, tag="ident")
            wq_r = wq.rearrange("(c p) m -> p c m", p=128)
            wk_r = wk.rearrange("(c p) m -> p c m", p=128)
            wv_r = wv.rearrange("(c p) m -> p c m", p=128)
            for g in range(8):
                gs = slice(g * 4, (g + 1) * 4)
                nc.gpsimd.dma_start(wk_sb[:, gs, :], wk_r[:, gs, :])
                nc.gpsimd.dma_start(wv_sb[:, gs, :], wv_r[:, gs, :])
                nc.gpsimd.dma_start(wq_sb[:, gs, :], wq_r[:, gs, :])
            nc.gpsimd.dma_start(cos_sb[:], cosT[:])
            nc.gpsimd.dma_start(sin_sb[:], sinT[:])
            make_identity(nc, ident[:])

            hsp = ph1.enter_context(tc.tile_pool(name="hs", bufs=6))
            pps = ph1.enter_context(
                tc.tile_pool(name="projps", bufs=6, space="PSUM"))
            rawp = ph1.enter_context(tc.tile_pool(name="raw", bufs=2))
            rotp = ph1.enter_context(tc.tile_pool(name="rot", bufs=2))
            tmpp = ph1.enter_context(tc.tile_pool(name="tmp", bufs=2))
            vtp = ph1.enter_context(tc.tile_pool(name="vtp", bufs=2))
            tps = ph1.enter_context(
                tc.tile_pool(name="tps", bufs=2, space="PSUM"))

            for n in range(NB):
                sl = slice(n * BLK, (n + 1) * BLK)
                ps = [pps.tile([128, BLK], fp32, tag="projps", name="projps")
                      for _ in range(G + 2)]
                for d in range(NDC):
                    hs_t = hsp.tile([128, BLK], f32r, tag="hs")
                    nc.sync.dma_start(hs_t[:], hsT[d * 128:(d + 1) * 128, sl])
                    for h in range(G):
                        nc.tensor.matmul(ps[h][:],
                                         wq_sb[:, d, h * DH:(h + 1) * DH],
                                         hs_t[:], start=(d == 0),
                                         stop=(d == NDC - 1))
                    nc.tensor.matmul(ps[G][:], wk_sb[:, d, :], hs_t[:],
                                     start=(d == 0), stop=(d == NDC - 1))
                    nc.tensor.matmul(ps[G + 1][:], wv_sb[:, d, :], hs_t[:],
                                     start=(d == 0), stop=(d == NDC - 1))
                # RoPE on q heads and k: evict PSUM, then rotate-half
                for i, dest in enumerate([qT[h][n] for h in range(G)]
                                         + [kT[n]]):
                    raw = rawp.tile([128, BLK], fp32, tag="raw")
                    nc.scalar.copy(raw[:], ps[i][:])
                    rot = rotp.tile([128, BLK], fp32, tag="rot")
                    nc.sync.dma_start(rot[0:64, :], raw[64:128, :])
                    nc.sync.dma_start(rot[64:128, :], raw[0:64, :])
                    tmp = tmpp.tile([128, BLK], fp32, tag="tmp")
                    nc.vector.tensor_mul(tmp[:], raw[:], cos_sb[:, sl])
                    nc.vector.tensor_mul(rot[:], rot[:], sin_sb[:, sl])
                    nc.vector.tensor_add(dest[:], tmp[:], rot[:])
                # V: evict then PE-transpose to [kcol, dh]
                vt = vtp.tile([128, BLK], fp32, tag="vt")
                nc.scalar.copy(vt[:], ps[G + 1][:])
                for j in range(BLK // 128):
                    c = n * (BLK // 128) + j
                    tp = tps.tile([128, DH], fp32, tag="tp")
                    nc.tensor.transpose(tp[:], vt[:, j * 128:(j + 1) * 128],
                                        ident[:])
                    nc.scalar.copy(vnat[c][:], tp[:])

        # -------- Phase 2+3: attention interleaved with output proj --------
        persist2 = top.enter_context(tc.tile_pool(name="persist2", bufs=1))
        attnT = [[persist2.tile([DH, BLK], f32r, tag=f"attnT{h}_{n}",
                                name=f"attnT{h}_{n}") for n in range(NB)]
                 for h in range(G)]
        wo_sb = persist2.tile([128, G, D], f32r, tag="wo", name="wo_sb")
        wo_r = wo.rearrange("(c p) n -> p c n", p=128)
        for g in range(8):
            nc.gpsimd.dma_start(wo_sb[:, :, g * BLK:(g + 1) * BLK],
                                wo_r[:, :, g * BLK:(g + 1) * BLK])
        with ExitStack() as ph2:
            c2 = ph2.enter_context(tc.tile_pool(name="p2const", bufs=1))
            ones = c2.tile([128, 1], fp32, tag="ones")
            nc.vector.memset(ones[:], 1.0)
            ones_r = c2.tile([128, 1], f32r, tag="ones_r")
            nc.vector.tensor_copy(ones_r[:], ones[:])
            wp = ph2.enter_context(tc.tile_pool(name="wp", bufs=4))
            tp2 = ph2.enter_context(tc.tile_pool(name="tp2", bufs=4))
            mp = ph2.enter_context(tc.tile_pool(name="mp", bufs=4))
            wsp = ph2.enter_context(tc.tile_pool(name="wsp", bufs=1))
            dsp = ph2.enter_context(tc.tile_pool(name="dsp", bufs=2))
            bcp = ph2.enter_context(tc.tile_pool(name="bcp", bufs=2))
            osb = ph2.enter_context(tc.tile_pool(name="osb", bufs=4))
            qkps = ph2.enter_context(
                tc.tile_pool(name="qkps", bufs=3, space="PSUM"))
            avps = ph2.enter_context(
                tc.tile_pool(name="avps", bufs=2, space="PSUM"))
            wops = ph2.enter_context(
                tc.tile_pool(name="wops", bufs=2, space="PSUM"))
            dnps = ph2.enter_context(
                tc.tile_pool(name="dnps", bufs=1, space="PSUM"))

            def wo_slice(s):
                n, j = s // (BLK // 128), s % (BLK // 128)
                for nn in range(D // BLK):
                    pso = wops.tile([128, BLK], fp32, tag="wop", name="wop")
                    for h in range(G):
                        nc.tensor.matmul(
                            pso[:], attnT[h][n][:, j * 128:(j + 1) * 128],
                            wo_sb[:, h, nn * BLK:(nn + 1) * BLK],
                            start=(h == 0), stop=(h == G - 1),
                            skip_group_check=True)
                    ot = osb.tile([128, BLK], fp32, tag="ot", name="ot")
                    nc.vector.tensor_copy(ot[:], pso[:])
                    nc.sync.dma_start(
                        out[s * 128:(s + 1) * 128,
                            nn * BLK:(nn + 1) * BLK], ot[:])

            for n in range(NB):
                mtl = {}
                for c, slot in active[n]:
                    if slot >= 0:
                        mt = mp.tile([128, BLK], fp32, tag="mask")
                        nc.sync.dma_start(
                            mt[:], maskm[slot * 128:(slot + 1) * 128, :])
                        mtl[c] = mt
                pairs = [active[n][i:i + 2]
                         for i in range(0, len(active[n]), 2)]
                n_mm = len(active[n])
                for h in range(G):
                    av = avps.tile([128, BLK], fp32, tag="av")
                    ws = wsp.tile([128, 2 * BLK], f32r, tag="wsum")
                    mm_i = 0
                    for j, pair in enumerate(pairs):
                        w2 = len(pair) * BLK
                        tt = tp2.tile([128, 2 * BLK], fp32, tag="tt")
                        for i, (c, slot) in enumerate(pair):
                            qk = qkps.tile([128, BLK], fp32, tag="qk")
                            nc.tensor.matmul(
                                qk[:],
                                kT[c // 4][:, (c % 4) * 128:(c % 4 + 1) * 128],
                                qT[h][n][:], start=True, stop=True)
                            nc.scalar.activation(
                                tt[:, i * BLK:(i + 1) * BLK], qk[:],
                                AF.Tanh, scale=1.0 / SOFTCAP)
                        wt = wp.tile([128, 2 * BLK], f32r, tag="wt")
                        nc.scalar.activation(wt[:, :w2], tt[:, :w2], AF.Exp,
                                             scale=SOFTCAP)
                        for i, (c, slot) in enumerate(pair):
                            if slot >= 0:
                                nc.vector.tensor_mul(
                                    wt[:, i * BLK:(i + 1) * BLK],
                                    wt[:, i * BLK:(i + 1) * BLK], mtl[c][:])
                        if j == 0:
                            nc.vector.tensor_copy(ws[:, :w2], wt[:, :w2])
                        else:
                            nc.vector.tensor_add(ws[:, :w2], ws[:, :w2],
                                                 wt[:, :w2])
                        for i, (c, slot) in enumerate(pair):
                            nc.tensor.matmul(av[:],
                                             vnat[c][:],
                                             wt[:, i * BLK:(i + 1) * BLK],
                                             start=(mm_i == 0),
                                             stop=(mm_i == n_mm - 1),
                                             skip_group_check=True)
                            mm_i += 1
                    # softmax denominator: ones-matmul over both halves
                    dn = dnps.tile([1, BLK], fp32, tag="dn")
                    if n_mm > 1:
                        nc.tensor.matmul(dn[:], ones_r[:], ws[:, :BLK],
                                         start=True, stop=False,
                                         skip_group_check=True)
                        nc.tensor.matmul(dn[:], ones_r[:], ws[:, BLK:],
                                         start=False, stop=True,
                                         skip_group_check=True)
                    else:
                        nc.tensor.matmul(dn[:], ones_r[:], ws[:, :BLK],
                                         start=True, stop=True)
                    dns = dsp.tile([1, BLK], fp32, tag="dns")
                    nc.vector.reciprocal(dns[:], dn[:])
                    bc = bcp.tile([128, BLK], fp32, tag="bc")
                    nc.gpsimd.partition_broadcast(bc[:], dns[:])
                    nc.vector.tensor_mul(attnT[h][n][:], av[:], bc[:])
                # output projection for this block's four row-slices
                for s in range(n * (BLK // 128), (n + 1) * (BLK // 128)):
                    wo_slice(s)

    nc.compile()
    return nc


def _rope_tables():
    j = np.arange(0, DH, 2, dtype=np.float32)
    inv = np.float32(1.0) / (np.float32(ROPE_BASE) ** (j / np.float32(DH)))
    t = np.arange(S, dtype=np.float32)
    phase = t[:, None] * inv[None, :]          # [S, 64] fp32 like reference
    cos = np.cos(phase).astype(np.float32)     # [S, 64]
    sin = np.sin(phase).astype(np.float32)
    cosT = np.concatenate([cos.T, cos.T], axis=0)              # [128, S]
    sinT = np.concatenate([-sin.T, sin.T], axis=0)             # sign-folded
    return np.ascontiguousarray(cosT), np.ascontiguousarray(sinT)


def _in_maps(hidden_states, mask, Wq, Wk, Wv, Wo):
    hs = np.asarray(hidden_states, dtype=np.float32).reshape(S, D)
    Wq = np.asarray(Wq, dtype=np.float32)
    Wk = np.asarray(Wk, dtype=np.float32)
    Wv = np.asarray(Wv, dtype=np.float32)
    Wo = np.asarray(Wo, dtype=np.float32)
    active, mt = _classify_mask(mask)
    hsT = np.ascontiguousarray(hs.T)
    cosT, sinT = _rope_tables()
    maps = []
    for c in range(N_CORES):
        m = {
            "hsT": hsT,
            "wq": np.ascontiguousarray(
                Wq[:, c * G * DH:(c + 1) * G * DH] * np.float32(MULT)),
            "wk": np.ascontiguousarray(Wk[:, c * DH:(c + 1) * DH]),
            "wv": np.ascontiguousarray(Wv[:, c * DH:(c + 1) * DH]),
            "wo": np.ascontiguousarray(Wo[c * G * DH:(c + 1) * G * DH, :]),
            "cosT": cosT,
            "sinT": sinT,
        }
        if mt is not None:
            m["maskm"] = mt
        maps.append(m)
    return active, mt, maps


def kernel(hidden_states, mask, Wq, Wk, Wv, Wo):
    from concourse.bass_utils import run_bass_kernel_spmd

    active, mt, maps = _in_maps(hidden_states, mask, Wq, Wk, Wv, Wo)
    key = tuple(tuple(lst) for lst in active)
    if key not in _CACHE:
        _CACHE[key] = _build(active, 0 if mt is None else mt.shape[0] // 128)
    nc = _CACHE[key]

    res = run_bass_kernel_spmd(nc, maps, list(range(N_CORES)))
    acc = np.zeros((S, D), dtype=np.float64)
    for c in range(N_CORES):
        acc += res.results[c]["out"]
    return acc.astype(np.float32).reshape(1, S, D)



# revision 4
# speedup vs baseline: 1.0958x; 1.0958x over previous
"""GQA multi-head attention (RoPE + tanh softcap + causal mask) on 8 TRN2 cores.

Sharding: tensor-parallel over the 8 kv-head groups (1 kv head + its 4 q heads
per core).  Each core computes its Q/K/V projections from the full hidden
states, runs attention for its 4 q heads, and produces a partial output
through its row-slice of Wo; the host sums the 8 partials.

v2 layout/schedule (vs the fp32 two-phase baseline):
  - all matmul operands are bf16 (PSUM accumulation stays fp32; softmax
    logits/tanh stay fp32).  Halves DMA + SBUF traffic and doubles DVE
    throughput on 16-bit elementwise work.  Measured end-to-end rel err
    ~4e-3 vs the 2e-2 gate.
  - single fused per-block pipeline: project block n (two 3-output passes
    over resident hs tiles) -> attention for q-block n over kv chunks
    0..n -> output projection rows of block n.  The tensor engine always
    has matmul work queued, so the HAM clock gate stays at 8/8 (the old
    kernel ran at 4/8 for 75% of its span).
  - softmax denominators accumulate on the PE: a per-chunk [1,512]
    ones-matmul rides the same PSUM accumulation pattern as A@V, replacing
    the serial vector-engine running-sum chain.
  - 1/denominator via the custom-DVE reciprocal_approx_fast (~5x faster
    than the 8-cycle/element iterative divide).
  - V tiles are transposed with the DMA crossbar (dma_start_transpose)
    instead of PE transposes, freeing PE time and a PSUM bank.
"""

import numpy as np

S, D, DH = 2048, 4096, 128
HQ, HKV = 32, 8
G = HQ // HKV            # q heads per core
N_CORES = 8
MULT = 0.08838834764831845
SOFTCAP = 30.0
ROPE_BASE = 10000.0
BLK = 512                # seq block
NB = S // BLK            # 4 seq blocks
NCH = S // 128           # 16 kcol chunks
NDC = D // 128           # 32 contraction chunks for projections
HDC = NDC // 2           # 16 d-chunks per hs half-block tile

_CACHE = {}


def _classify_mask(mask):
    """Per (qblock, kchunk): skip (all masked), plain (all visible), or
    mixed (transposed {0,1} tile, deduped).  active[n] = ordered
    [(chunk, slot)], slot -1 for plain; mtiles packed [n_uniq*128, BLK]."""
    m = np.asarray(mask).reshape(S, S)
    active = []
    mtiles = []
    seen = {}
    for n in range(NB):
        rows = m[n * BLK:(n + 1) * BLK]
        lst = []
        for c in range(NCH):
            sub = rows[:, c * 128:(c + 1) * 128]
            if not sub.any():
                continue
            if sub.all():
                lst.append((c, -1))
            else:
                t = np.ascontiguousarray(sub.T).astype(np.float32)
                key = t.tobytes()
                if key not in seen:
                    seen[key] = len(mtiles)
                    mtiles.append(t)
                lst.append((c, seen[key]))
        active.append(lst)
    mt = (np.concatenate([t.reshape(128, BLK) for t in mtiles], axis=0)
          if mtiles else None)
    return active, mt


def _build(active, n_uniq):
    import concourse.bacc as bacc
    import concourse.mybir as mybir
    from concourse import tile
    from contextlib import ExitStack

    fp32 = mybir.dt.float32
    bf16 = mybir.dt.bfloat16
    AF = mybir.ActivationFunctionType

    nc = bacc.Bacc("TRN2", target_bir_lowering=False, debug=False,
                   enable_asserts=True, num_devices=N_CORES)
    hsT = nc.dram_tensor("hsT", [D, S], bf16, kind="ExternalInput").ap()
    wq = nc.dram_tensor("wq", [D, G * DH], bf16, kind="ExternalInput").ap()
    wk = nc.dram_tensor("wk", [D, DH], bf16, kind="ExternalInput").ap()
    wv = nc.dram_tensor("wv", [D, DH], bf16, kind="ExternalInput").ap()
    wo = nc.dram_tensor("wo", [G * DH, D], bf16, kind="ExternalInput").ap()
    cosT = nc.dram_tensor("cosT", [DH, S], bf16, kind="ExternalInput").ap()
    sinT = nc.dram_tensor("sinT", [DH, S], bf16, kind="ExternalInput").ap()
    maskm = (nc.dram_tensor("maskm", [n_uniq * 128, BLK], bf16,
                            kind="ExternalInput").ap() if n_uniq else None)
    out = nc.dram_tensor("out", [S, D], fp32, kind="ExternalOutput").ap()

    hsT_r = hsT.rearrange("(c p) s -> p c s", p=128)
    wq_r = wq.rearrange("(c p) m -> p c m", p=128)
    wk_r = wk.rearrange("(c p) m -> p c m", p=128)
    wv_r = wv.rearrange("(c p) m -> p c m", p=128)
    wo_r = wo.rearrange("(c p) n -> p c n", p=128)

    with tile.TileContext(nc) as tc, ExitStack() as top:
        persist = top.enter_context(tc.tile_pool(name="persist", bufs=1))
        # weights: per-head wq tiles so the first matmuls don't wait on the
        # whole 4MB load
        wq_sb = [persist.tile([128, NDC, DH], bf16, tag=f"wq{h}",
                              name=f"wq{h}") for h in range(G)]
        wk_sb = persist.tile([128, NDC, DH], bf16, tag="wk", name="wk")
        wv_sb = persist.tile([128, NDC, DH], bf16, tag="wv", name="wv")
        wo_sb = persist.tile([128, G, D], bf16, tag="wo", name="wo_sb")
        cos_sb = persist.tile([DH, S], bf16, tag="cos", name="cos")
        sin_sb = persist.tile([DH, S], bf16, tag="sin", name="sin")
        ones_bf = persist.tile([128, 1], bf16, tag="ones", name="ones")
        mask_sb = (persist.tile([128, n_uniq, BLK], bf16, tag="masks",
                                name="masks") if n_uniq else None)
        qT = [[persist.tile([DH, BLK], bf16, tag=f"qT{h}_{n}",
                            name=f"qT{h}_{n}") for n in range(NB)]
              for h in range(G)]
        kT = [persist.tile([DH, BLK], bf16, tag=f"kT{n}", name=f"kT{n}")
              for n in range(NB)]
        vnat = [persist.tile([128, DH], bf16, tag=f"vnat{c}", name=f"vnat{c}")
                for c in range(NCH)]
        attnT = [[persist.tile([DH, BLK], bf16, tag=f"attnT{h}_{n}",
                               name=f"attnT{h}_{n}") for n in range(NB)]
                 for h in range(G)]

        # weight loads: pass-A weights first so projections start early
        for h in range(G):
            nc.gpsimd.dma_start(wq_sb[h][:], wq_r[:, :, h * DH:(h + 1) * DH])
        nc.gpsimd.dma_start(wk_sb[:], wk_r[:])
        nc.gpsimd.dma_start(wv_sb[:], wv_r[:])
        nc.gpsimd.dma_start(cos_sb[:], cosT[:])
        nc.gpsimd.dma_start(sin_sb[:], sinT[:])
        nc.vector.memset(ones_bf[:], 1.0)
        if n_uniq:
            mm_r = maskm.rearrange("(u p) s -> p u s", p=128)
            nc.gpsimd.dma_start(mask_sb[:], mm_r[:])
        for g in range(8):
            nc.gpsimd.dma_start(wo_sb[:, :, g * BLK:(g + 1) * BLK],
                                wo_r[:, :, g * BLK:(g + 1) * BLK])

        # working pools
        hsp = top.enter_context(tc.tile_pool(name="hs", bufs=2))
        pps = top.enter_context(tc.tile_pool(name="projps", bufs=3,
                                             space="PSUM"))
        rawp = top.enter_context(tc.tile_pool(name="raw", bufs=2))
        rotp = top.enter_context(tc.tile_pool(name="rot", bufs=2))
        tmpp = top.enter_context(tc.tile_pool(name="tmp", bufs=2))
        vtp = top.enter_context(tc.tile_pool(name="vt", bufs=2))
        ttp = top.enter_context(tc.tile_pool(name="tt", bufs=2))
        wtp = top.enter_context(tc.tile_pool(name="wt", bufs=2))
        dsp = top.enter_context(tc.tile_pool(name="dns", bufs=2))
        bcp = top.enter_context(tc.tile_pool(name="bc", bufs=2))
        osb = top.enter_context(tc.tile_pool(name="osb", bufs=3))
        qkps = top.enter_context(tc.tile_pool(name="qkps", bufs=1,
                                              space="PSUM"))
        avps = top.enter_context(tc.tile_pool(name="avps", bufs=1,
                                              space="PSUM"))
        dnps = top.enter_context(tc.tile_pool(name="dnps", bufs=1,
                                              space="PSUM"))
        wops = top.enter_context(tc.tile_pool(name="wops", bufs=2,
                                              space="PSUM"))

        def rope_evict(ps, dest, sl):
            raw = rawp.tile([128, BLK], bf16, tag="raw", name="raw")
            nc.scalar.copy(raw[:], ps[:])
            rot = rotp.tile([128, BLK], bf16, tag="rot", name="rot")
            nc.sync.dma_start(rot[0:64, :], raw[64:128, :])
            nc.sync.dma_start(rot[64:128, :], raw[0:64, :])
            tmp = tmpp.tile([128, BLK], bf16, tag="tmp", name="tmp")
            nc.vector.tensor_mul(tmp[:], raw[:], cos_sb[:, sl])
            nc.vector.tensor_mul(rot[:], rot[:], sin_sb[:, sl])
            nc.vector.tensor_add(dest[:], tmp[:], rot[:])

        for n in range(NB):
            sl = slice(n * BLK, (n + 1) * BLK)
            hs_t = [hsp.tile([128, HDC, BLK], bf16, tag="hs", name="hs_t")
                    for _ in range(2)]
            for t in range(2):
                nc.sync.dma_start(hs_t[t][:],
                                  hsT_r[:, t * HDC:(t + 1) * HDC, sl])

            # pass A: q heads 0-2
            psA = [pps.tile([128, BLK], fp32, tag="projps", name="projps")
                   for _ in range(3)]
            for d in range(NDC):
                h_ap = hs_t[d // HDC][:, d % HDC, :]
                for i in range(3):
                    nc.tensor.matmul(psA[i][:],
                                     wq_sb[i][:, d, :], h_ap,
                                     start=(d == 0), stop=(d == NDC - 1))
            for i in range(3):
                rope_evict(psA[i], qT[i][n], sl)

            # pass B: q head 3, k, v
            psB = [pps.tile([128, BLK], fp32, tag="projps", name="projps")
                   for _ in range(3)]
            wB = [wq_sb[3], wk_sb, wv_sb]
            for d in range(NDC):
                h_ap = hs_t[d // HDC][:, d % HDC, :]
                for i in range(3):
                    nc.tensor.matmul(psB[i][:],
                                     wB[i][:, d, :], h_ap,
                                     start=(d == 0), stop=(d == NDC - 1))
            rope_evict(psB[0], qT[3][n], sl)
            rope_evict(psB[1], kT[n], sl)
            # V: evict to bf16, transpose chunks with the DMA crossbar
            vt = vtp.tile([128, BLK], bf16, tag="vt", name="vt")
            nc.scalar.copy(vt[:], psB[2][:])
            for j in range(BLK // 128):
                c = n * (BLK // 128) + j
                nc.sync.dma_start_transpose(
                    vnat[c][:], vt[:, j * 128:(j + 1) * 128])

            # ---- attention for q-block n over kv chunks 0..4n+3 ----
            pairs = [active[n][i:i + 2] for i in range(0, len(active[n]), 2)]
            n_mm = len(active[n])
            for h in range(G):
                av = avps.tile([128, BLK], fp32, tag="av", name="av")
                dn = dnps.tile([1, BLK], fp32, tag="dn", name="dn")
                mm_i = 0
                for pair in pairs:
                    w2 = len(pair) * BLK
                    tt = ttp.tile([128, 2 * BLK], fp32, tag="tt", name="tt")
                    for i, (c, slot) in enumerate(pair):
                        qk = qkps.tile([128, BLK], fp32, tag="qk", name="qk")
                        nc.tensor.matmul(
                            qk[:],
                            kT[c // 4][:, (c % 4) * 128:(c % 4 + 1) * 128],
                            qT[h][n][:], start=True, stop=True)
                        nc.scalar.activation(
                            tt[:, i * BLK:(i + 1) * BLK], qk[:],
                            AF.Tanh, scale=1.0 / SOFTCAP)
                    wt = wtp.tile([128, 2 * BLK], bf16, tag="wt", name="wt")
                    nc.scalar.activation(wt[:, :w2], tt[:, :w2], AF.Exp,
                                         scale=SOFTCAP)
                    for i, (c, slot) in enumerate(pair):
                        if slot >= 0:
                            nc.vector.tensor_mul(
                                wt[:, i * BLK:(i + 1) * BLK],
                                wt[:, i * BLK:(i + 1) * BLK],
                                mask_sb[:, slot, :])
                    for i, (c, slot) in enumerate(pair):
                        wt_ap = wt[:, i * BLK:(i + 1) * BLK]
                        nc.tensor.matmul(av[:], vnat[c][:], wt_ap,
                                         start=(mm_i == 0),
                                         stop=(mm_i == n_mm - 1),
                                         skip_group_check=True)
                        nc.tensor.matmul(dn[:], ones_bf[:], wt_ap,
                                         start=(mm_i == 0),
                                         stop=(mm_i == n_mm - 1),
                                         skip_group_check=True)
                        mm_i += 1
                dns = dsp.tile([1, BLK], fp32, tag="dns", name="dns")
                nc.vector.reciprocal_approx_fast(dns[:], dn[:])
                bc = bcp.tile([128, BLK], fp32, tag="bc", name="bc")
                nc.gpsimd.partition_broadcast(bc[:], dns[:])
                nc.vector.tensor_mul(attnT[h][n][:], av[:], bc[:])

            # ---- output projection for this block's four row-slices ----
            for j in range(BLK // 128):
                s = n * (BLK // 128) + j
                for nn2 in range(D // (2 * BLK)):
                    ot = osb.tile([128, 2 * BLK], fp32, tag="ot", name="ot")
                    for half in range(2):
                        nn = nn2 * 2 + half
                        pso = wops.tile([128, BLK], fp32, tag="wop",
                                        name="wop")
                        for h in range(G):
                            nc.tensor.matmul(
                                pso[:], attnT[h][n][:, j * 128:(j + 1) * 128],
                                wo_sb[:, h, nn * BLK:(nn + 1) * BLK],
                                start=(h == 0), stop=(h == G - 1),
                                skip_group_check=True)
                        nc.vector.tensor_copy(
                            ot[:, half * BLK:(half + 1) * BLK], pso[:])
                    nc.gpsimd.dma_start(
                        out[s * 128:(s + 1) * 128,
                            nn2 * 2 * BLK:(nn2 + 1) * 2 * BLK], ot[:])

    nc.compile()
    return nc


def _rope_tables():
    j = np.arange(0, DH, 2, dtype=np.float32)
    inv = np.float32(1.0) / (np.float32(ROPE_BASE) ** (j / np.float32(DH)))
    t = np.arange(S, dtype=np.float32)
    phase = t[:, None] * inv[None, :]          # [S, 64] fp32 like reference
    cos = np.cos(phase).astype(np.float32)     # [S, 64]
    sin = np.sin(phase).astype(np.float32)
    cosT = np.concatenate([cos.T, cos.T], axis=0)              # [128, S]
    sinT = np.concatenate([-sin.T, sin.T], axis=0)             # sign-folded
    return np.ascontiguousarray(cosT), np.ascontiguousarray(sinT)


def _in_maps(hidden_states, mask, Wq, Wk, Wv, Wo):
    import ml_dtypes
    bf = ml_dtypes.bfloat16
    hs = np.asarray(hidden_states, dtype=np.float32).reshape(S, D)
    Wq = np.asarray(Wq, dtype=np.float32)
    Wk = np.asarray(Wk, dtype=np.float32)
    Wv = np.asarray(Wv, dtype=np.float32)
    Wo = np.asarray(Wo, dtype=np.float32)
    active, mt = _classify_mask(mask)
    hsT = np.ascontiguousarray(hs.T.astype(bf))
    cosT, sinT = _rope_tables()
    cosT = cosT.astype(bf)
    sinT = sinT.astype(bf)
    maps = []
    for c in range(N_CORES):
        m = {
            "hsT": hsT,
            "wq": np.ascontiguousarray(
                (Wq[:, c * G * DH:(c + 1) * G * DH]
                 * np.float32(MULT)).astype(bf)),
            "wk": np.ascontiguousarray(Wk[:, c * DH:(c + 1) * DH].astype(bf)),
            "wv": np.ascontiguousarray(Wv[:, c * DH:(c + 1) * DH].astype(bf)),
            "wo": np.ascontiguousarray(
                Wo[c * G * DH:(c + 1) * G * DH, :].astype(bf)),
            "cosT": cosT,
            "sinT": sinT,
        }
        if mt is not None:
            m["maskm"] = np.ascontiguousarray(mt.astype(bf))
        maps.append(m)
    return active, mt, maps


def kernel(hidden_states, mask, Wq, Wk, Wv, Wo):
    from concourse.bass_utils import run_bass_kernel_spmd

    active, mt, maps = _in_maps(hidden_states, mask, Wq, Wk, Wv, Wo)
    key = tuple(tuple(lst) for lst in active)
    if key not in _CACHE:
        _CACHE[key] = _build(active, 0 if mt is None else mt.shape[0] // 128)
    nc = _CACHE[key]

    res = run_bass_kernel_spmd(nc, maps, list(range(N_CORES)))
    acc = np.zeros((S, D), dtype=np.float64)
    for c in range(N_CORES):
        acc += res.results[c]["out"]
    return acc.astype(np.float32).reshape(1, S, D)


# revision 6
# speedup vs baseline: 1.2644x; 1.1539x over previous
"""GQA multi-head attention (RoPE + tanh softcap + causal mask) on 8 TRN2 cores.

Sharding: tensor-parallel over the 8 kv-head groups (1 kv head + its 4 q heads
per core).  Each core computes its Q/K/V projections from the full hidden
states, runs attention for its 4 q heads, and produces a partial output
through its row-slice of Wo; the host sums the 8 partials.

v2 layout/schedule (vs the fp32 two-phase baseline):
  - all matmul operands are bf16 (PSUM accumulation stays fp32; softmax
    logits/tanh stay fp32).  Halves DMA + SBUF traffic and doubles DVE
    throughput on 16-bit elementwise work.  Measured end-to-end rel err
    ~4e-3 vs the 2e-2 gate.
  - single fused per-block pipeline: project block n (two 3-output passes
    over resident hs tiles) -> attention for q-block n over kv chunks
    0..n -> output projection rows of block n.  The tensor engine always
    has matmul work queued, so the HAM clock gate stays at 8/8 (the old
    kernel ran at 4/8 for 75% of its span).
  - softmax denominators accumulate on the PE: a per-chunk [1,512]
    ones-matmul rides the same PSUM accumulation pattern as A@V, replacing
    the serial vector-engine running-sum chain.
  - 1/denominator via the custom-DVE reciprocal_approx_fast (~5x faster
    than the 8-cycle/element iterative divide).
  - V tiles are transposed with the DMA crossbar (dma_start_transpose)
    instead of PE transposes, freeing PE time and a PSUM bank.
"""

import numpy as np

S, D, DH = 2048, 4096, 128
HQ, HKV = 32, 8
G = HQ // HKV            # q heads per core
N_CORES = 8
MULT = 0.08838834764831845
SOFTCAP = 30.0
ROPE_BASE = 10000.0
BLK = 512                # seq block
NB = S // BLK            # 4 seq blocks
NCH = S // 128           # 16 kcol chunks
NDC = D // 128           # 32 contraction chunks for projections
HDC = NDC // 2           # 16 d-chunks per hs half-block tile

_CACHE = {}


def _classify_mask(mask):
    """Per (qblock, kchunk): skip (all masked), plain (all visible), or
    mixed (transposed {0,1} tile, deduped).  active[n] = ordered
    [(chunk, slot)], slot -1 for plain; mtiles packed [n_uniq*128, BLK]."""
    m = np.asarray(mask).reshape(S, S)
    active = []
    mtiles = []
    seen = {}
    for n in range(NB):
        rows = m[n * BLK:(n + 1) * BLK]
        lst = []
        for c in range(NCH):
            sub = rows[:, c * 128:(c + 1) * 128]
            if not sub.any():
                continue
            if sub.all():
                lst.append((c, -1))
            else:
                t = np.ascontiguousarray(sub.T).astype(np.float32)
                key = t.tobytes()
                if key not in seen:
                    seen[key] = len(mtiles)
                    mtiles.append(t)
                lst.append((c, seen[key]))
        active.append(lst)
    mt = (np.concatenate([t.reshape(128, BLK) for t in mtiles], axis=0)
          if mtiles else None)
    return active, mt


def _build(active, n_uniq):
    import concourse.bacc as bacc
    import concourse.mybir as mybir
    from concourse import tile
    from contextlib import ExitStack

    fp32 = mybir.dt.float32
    bf16 = mybir.dt.bfloat16
    AF = mybir.ActivationFunctionType

    nc = bacc.Bacc("TRN2", target_bir_lowering=False, debug=False,
                   enable_asserts=True, num_devices=N_CORES)
    hsT = nc.dram_tensor("hsT", [D, S], bf16, kind="ExternalInput").ap()
    wq = nc.dram_tensor("wq", [D, G * DH], bf16, kind="ExternalInput").ap()
    wk = nc.dram_tensor("wk", [D, DH], bf16, kind="ExternalInput").ap()
    wv = nc.dram_tensor("wv", [D, DH], bf16, kind="ExternalInput").ap()
    wo = nc.dram_tensor("wo", [G * DH, D], bf16, kind="ExternalInput").ap()
    cosT = nc.dram_tensor("cosT", [DH, S], bf16, kind="ExternalInput").ap()
    sinT = nc.dram_tensor("sinT", [DH, S], bf16, kind="ExternalInput").ap()
    maskm = (nc.dram_tensor("maskm", [n_uniq * 128, BLK], bf16,
                            kind="ExternalInput").ap() if n_uniq else None)
    out = nc.dram_tensor("out", [S, D], fp32, kind="ExternalOutput").ap()

    hsT_r = hsT.rearrange("(c p) s -> p c s", p=128)
    wq_r = wq.rearrange("(c p) m -> p c m", p=128)
    wk_r = wk.rearrange("(c p) m -> p c m", p=128)
    wv_r = wv.rearrange("(c p) m -> p c m", p=128)
    wo_r = wo.rearrange("(c p) n -> p c n", p=128)

    with tile.TileContext(nc) as tc, ExitStack() as top:
        persist = top.enter_context(tc.tile_pool(name="persist", bufs=1))
        # weights: per-head wq tiles so the first matmuls don't wait on the
        # whole 4MB load
        wq_sb = [persist.tile([128, NDC, DH], bf16, tag=f"wq{h}",
                              name=f"wq{h}") for h in range(G)]
        wk_sb = persist.tile([128, NDC, DH], bf16, tag="wk", name="wk")
        wv_sb = persist.tile([128, NDC, DH], bf16, tag="wv", name="wv")
        wo_sb = persist.tile([128, G, D], bf16, tag="wo", name="wo_sb")
        cos_sb = persist.tile([DH, S], bf16, tag="cos", name="cos")
        sin_sb = persist.tile([DH, S], bf16, tag="sin", name="sin")
        ones_bf = persist.tile([128, 1], bf16, tag="ones", name="ones")
        mask_sb = (persist.tile([128, n_uniq, BLK], bf16, tag="masks",
                                name="masks") if n_uniq else None)
        qT = [[persist.tile([DH, BLK], bf16, tag=f"qT{h}_{n}",
                            name=f"qT{h}_{n}") for n in range(NB)]
              for h in range(G)]
        kT = [persist.tile([DH, BLK], bf16, tag=f"kT{n}", name=f"kT{n}")
              for n in range(NB)]
        vnat = [persist.tile([128, DH], bf16, tag=f"vnat{c}", name=f"vnat{c}")
                for c in range(NCH)]
        attnT = [[persist.tile([DH, BLK], bf16, tag=f"attnT{h}_{n}",
                               name=f"attnT{h}_{n}") for n in range(NB)]
                 for h in range(G)]

        # weight loads on the HWDGE queues (sync+scalar), in first-use order
        for h in range(2):
            nc.sync.dma_start(wq_sb[h][:], wq_r[:, :, h * DH:(h + 1) * DH])
        for h in range(2, G):
            nc.scalar.dma_start(wq_sb[h][:], wq_r[:, :, h * DH:(h + 1) * DH])
        nc.sync.dma_start(wk_sb[:], wk_r[:])
        nc.sync.dma_start(wv_sb[:], wv_r[:])
        nc.scalar.dma_start(cos_sb[:], cosT[:])
        nc.scalar.dma_start(sin_sb[:], sinT[:])
        nc.vector.memset(ones_bf[:], 1.0)
        if n_uniq:
            mm_r = maskm.rearrange("(u p) s -> p u s", p=128)
            nc.sync.dma_start(mask_sb[:], mm_r[:])
        for g in range(8):
            nc.gpsimd.dma_start(wo_sb[:, :, g * BLK:(g + 1) * BLK],
                                wo_r[:, :, g * BLK:(g + 1) * BLK])

        # working pools
        hsp = top.enter_context(tc.tile_pool(name="hs", bufs=2))
        pps = top.enter_context(tc.tile_pool(name="projps", bufs=2,
                                             space="PSUM"))
        rawp = top.enter_context(tc.tile_pool(name="raw", bufs=2))
        rotp = top.enter_context(tc.tile_pool(name="rot", bufs=2))
        tmpp = top.enter_context(tc.tile_pool(name="tmp", bufs=2))
        vtp = top.enter_context(tc.tile_pool(name="vt", bufs=2))
        ttp = top.enter_context(tc.tile_pool(name="tt", bufs=2))
        wtp = top.enter_context(tc.tile_pool(name="wt", bufs=2))
        dsp = top.enter_context(tc.tile_pool(name="dns", bufs=2))
        bcp = top.enter_context(tc.tile_pool(name="bc", bufs=2))
        osb = top.enter_context(tc.tile_pool(name="osb", bufs=3))
        qkps = top.enter_context(tc.tile_pool(name="qkps", bufs=2,
                                              space="PSUM"))
        avps = top.enter_context(tc.tile_pool(name="avps", bufs=2,
                                              space="PSUM"))
        dnps = top.enter_context(tc.tile_pool(name="dnps", bufs=1,
                                              space="PSUM"))
        wops = top.enter_context(tc.tile_pool(name="wops", bufs=1,
                                              space="PSUM"))

        def wo_block(n):
            for j in range(BLK // 128):
                s = n * (BLK // 128) + j
                for nn2 in range(D // (2 * BLK)):
                    ot = osb.tile([128, 2 * BLK], fp32, tag="ot", name="ot")
                    for half in range(2):
                        nn = nn2 * 2 + half
                        pso = wops.tile([128, BLK], fp32, tag="wop",
                                        name="wop")
                        for h in range(G):
                            nc.tensor.matmul(
                                pso[:], attnT[h][n][:, j * 128:(j + 1) * 128],
                                wo_sb[:, h, nn * BLK:(nn + 1) * BLK],
                                start=(h == 0), stop=(h == G - 1),
                                skip_group_check=True)
                        nc.vector.tensor_copy(
                            ot[:, half * BLK:(half + 1) * BLK], pso[:])
                    nc.gpsimd.dma_start(
                        out[s * 128:(s + 1) * 128,
                            nn2 * 2 * BLK:(nn2 + 1) * 2 * BLK], ot[:])

        def rope_evict(ps, dest, sl):
            raw = rawp.tile([128, BLK], bf16, tag="raw", name="raw")
            nc.scalar.copy(raw[:], ps[:])
            rot = rotp.tile([128, BLK], bf16, tag="rot", name="rot")
            nc.sync.dma_start(rot[0:64, :], raw[64:128, :])
            nc.sync.dma_start(rot[64:128, :], raw[0:64, :])
            tmp = tmpp.tile([128, BLK], bf16, tag="tmp", name="tmp")
            nc.vector.tensor_mul(tmp[:], raw[:], cos_sb[:, sl])
            nc.vector.tensor_mul(rot[:], rot[:], sin_sb[:, sl])
            nc.vector.tensor_add(dest[:], tmp[:], rot[:])

        for n in range(NB):
            sl = slice(n * BLK, (n + 1) * BLK)
            hs_t = [hsp.tile([128, HDC, BLK], bf16, tag="hs", name="hs_t")
                    for _ in range(2)]
            for t in range(2):
                nc.sync.dma_start(hs_t[t][:],
                                  hsT_r[:, t * HDC:(t + 1) * HDC, sl])

            # three projection passes of two outputs each (2 PSUM banks)
            passes = [(wq_sb[0], qT[0][n], wq_sb[1], qT[1][n]),
                      (wq_sb[2], qT[2][n], wq_sb[3], qT[3][n]),
                      (wk_sb, kT[n], wv_sb, None)]
            for w0, d0, w1, d1 in passes:
                ps0 = pps.tile([128, BLK], fp32, tag="projps", name="projps")
                ps1 = pps.tile([128, BLK], fp32, tag="projps", name="projps")
                for d in range(NDC):
                    h_ap = hs_t[d // HDC][:, d % HDC, :]
                    nc.tensor.matmul(ps0[:], w0[:, d, :], h_ap,
                                     start=(d == 0), stop=(d == NDC - 1))
                    nc.tensor.matmul(ps1[:], w1[:, d, :], h_ap,
                                     start=(d == 0), stop=(d == NDC - 1))
                rope_evict(ps0, d0, sl)
                if d1 is not None:
                    rope_evict(ps1, d1, sl)
                else:
                    # V: evict to bf16, transpose chunks with the DMA crossbar
                    vt = vtp.tile([128, BLK], bf16, tag="vt", name="vt")
                    nc.scalar.copy(vt[:], ps1[:])
                    for j in range(BLK // 128):
                        c = n * (BLK // 128) + j
                        nc.sync.dma_start_transpose(
                            vnat[c][:], vt[:, j * 128:(j + 1) * 128])

            # ---- attention for q-block n over kv chunks 0..4n+3 ----
            pairs = [active[n][i:i + 2] for i in range(0, len(active[n]), 2)]
            n_mm = len(active[n])
            for h in range(G):
                av = avps.tile([128, BLK], fp32, tag="av", name="av")
                dn = dnps.tile([1, BLK], fp32, tag="dn", name="dn")
                mm_i = 0
                for pair in pairs:
                    w2 = len(pair) * BLK
                    tt = ttp.tile([128, 2 * BLK], fp32, tag="tt", name="tt")
                    for i, (c, slot) in enumerate(pair):
                        qk = qkps.tile([128, BLK], fp32, tag="qk", name="qk")
                        nc.tensor.matmul(
                            qk[:],
                            kT[c // 4][:, (c % 4) * 128:(c % 4 + 1) * 128],
                            qT[h][n][:], start=True, stop=True)
                        nc.scalar.activation(
                            tt[:, i * BLK:(i + 1) * BLK], qk[:],
                            AF.Tanh, scale=1.0 / SOFTCAP)
                    wt = wtp.tile([128, 2 * BLK], bf16, tag="wt", name="wt")
                    nc.scalar.activation(wt[:, :w2], tt[:, :w2], AF.Exp,
                                         scale=SOFTCAP)
                    for i, (c, slot) in enumerate(pair):
                        if slot >= 0:
                            nc.vector.tensor_mul(
                                wt[:, i * BLK:(i + 1) * BLK],
                                wt[:, i * BLK:(i + 1) * BLK],
                                mask_sb[:, slot, :])
                    for i, (c, slot) in enumerate(pair):
                        wt_ap = wt[:, i * BLK:(i + 1) * BLK]
                        nc.tensor.matmul(av[:], vnat[c][:], wt_ap,
                                         start=(mm_i == 0),
                                         stop=(mm_i == n_mm - 1),
                                         skip_group_check=True)
                        nc.tensor.matmul(dn[:], ones_bf[:], wt_ap,
                                         start=(mm_i == 0),
                                         stop=(mm_i == n_mm - 1),
                                         skip_group_check=True)
                        mm_i += 1
                dns = dsp.tile([1, BLK], fp32, tag="dns", name="dns")
                nc.vector.reciprocal_approx_fast(dns[:], dn[:])
                bc = bcp.tile([128, BLK], fp32, tag="bc", name="bc")
                nc.gpsimd.partition_broadcast(bc[:], dns[:])
                nc.vector.tensor_mul(attnT[h][n][:], av[:], bc[:])

            # output projection of the PREVIOUS block: emitted after this
            # block's attention so its matmuls fill attention pipeline gaps
            if n > 0:
                wo_block(n - 1)
        wo_block(NB - 1)

    nc.compile()
    return nc


def _rope_tables():
    j = np.arange(0, DH, 2, dtype=np.float32)
    inv = np.float32(1.0) / (np.float32(ROPE_BASE) ** (j / np.float32(DH)))
    t = np.arange(S, dtype=np.float32)
    phase = t[:, None] * inv[None, :]          # [S, 64] fp32 like reference
    cos = np.cos(phase).astype(np.float32)     # [S, 64]
    sin = np.sin(phase).astype(np.float32)
    cosT = np.concatenate([cos.T, cos.T], axis=0)              # [128, S]
    sinT = np.concatenate([-sin.T, sin.T], axis=0)             # sign-folded
    return np.ascontiguousarray(cosT), np.ascontiguousarray(sinT)


def _in_maps(hidden_states, mask, Wq, Wk, Wv, Wo):
    import ml_dtypes
    bf = ml_dtypes.bfloat16
    hs = np.asarray(hidden_states, dtype=np.float32).reshape(S, D)
    Wq = np.asarray(Wq, dtype=np.float32)
    Wk = np.asarray(Wk, dtype=np.float32)
    Wv = np.asarray(Wv, dtype=np.float32)
    Wo = np.asarray(Wo, dtype=np.float32)
    active, mt = _classify_mask(mask)
    hsT = np.ascontiguousarray(hs.T.astype(bf))
    cosT, sinT = _rope_tables()
    cosT = cosT.astype(bf)
    sinT = sinT.astype(bf)
    maps = []
    for c in range(N_CORES):
        m = {
            "hsT": hsT,
            "wq": np.ascontiguousarray(
                (Wq[:, c * G * DH:(c + 1) * G * DH]
                 * np.float32(MULT)).astype(bf)),
            "wk": np.ascontiguousarray(Wk[:, c * DH:(c + 1) * DH].astype(bf)),
            "wv": np.ascontiguousarray(Wv[:, c * DH:(c + 1) * DH].astype(bf)),
            "wo": np.ascontiguousarray(
                Wo[c * G * DH:(c + 1) * G * DH, :].astype(bf)),
            "cosT": cosT,
            "sinT": sinT,
        }
        if mt is not None:
            m["maskm"] = np.ascontiguousarray(mt.astype(bf))
        maps.append(m)
    return active, mt, maps


def kernel(hidden_states, mask, Wq, Wk, Wv, Wo):
    from concourse.bass_utils import run_bass_kernel_spmd

    active, mt, maps = _in_maps(hidden_states, mask, Wq, Wk, Wv, Wo)
    key = tuple(tuple(lst) for lst in active)
    if key not in _CACHE:
        _CACHE[key] = _build(active, 0 if mt is None else mt.shape[0] // 128)
    nc = _CACHE[key]

    res = run_bass_kernel_spmd(nc, maps, list(range(N_CORES)))
    acc = np.zeros((S, D), dtype=np.float64)
    for c in range(N_CORES):
        acc += res.results[c]["out"]
    return acc.astype(np.float32).reshape(1, S, D)
